# revision 13
# baseline (speedup 1.0000x reference)
"""Segment-prefix max kernel for Trainium2 (8 NeuronCores, SPMD).

Problem: x [1048576, 128] f32, 2048 uniform segments of 512 rows each;
out[i, :] = max over the first (512 - window_size + 1) rows of segment i.

Strategy (memory-bound; reads the near-minimal ~488 MiB of HBM):
  - Shard segments across 8 cores: core c gets rows [c*131072, (c+1)*131072)
    and produces out rows [c*256, (c+1)*256). No cross-core communication.
  - Per core, 16 tiles of 16 segments. Each segment contributes its first
    15 runs of 32 rows (16 KiB contiguous DMA runs); run 15 (which contains
    the invalid window tail) is NOT loaded. Partition 15*l + i holds run i
    of fill-segment l, partitions 120..127 idle. 8 plain-sliced DMAs per
    tile alternate between the sync and scalar HWDGE rings.
  - The valid tail rows [count-32, count) of all segments are loaded after
    the tile loop (2 x 2 MiB), folded, and joined with one final max.
  - The 32 -> 1 fold along the free axis runs on DVE as a binary tree; the
    first level reads f32 and writes bf16, middle levels run in bf16 at 2x
    DVE throughput, the last level emits f32 (rel tolerance 2e-2 >> bf16's
    ~4e-3 rounding).
  - Cross-partition max (each segment = 15 consecutive partitions of one
    fill) goes through a PE transpose (identity matmul into PSUM) and one
    DVE reduce_max per fill into a [128 d, n_seg] column accumulator.
  - Final columns are PE-transposed back to row-major [n_seg, 128] chunks
    and DMA'd out.
"""

import numpy as np

import concourse.bacc as bacc
import concourse.bass as bass
import concourse.tile as tile
from concourse import mybir
from concourse.bass_utils import run_bass_kernel_spmd
from concourse.masks import make_identity

N_CORES = 8
SEG_LEN = 512
D = 128
J = 32  # rows per run (16 KiB contiguous DMA run)
RUNS = SEG_LEN // J  # 16 runs per segment (run 15 not loaded in bulk)
BULK = RUNS - 1  # 15 bulk runs per segment
FILLS = 2  # fills per tile
SEGS_PER_FILL = 8
SEGS_PER_TILE = FILLS * SEGS_PER_FILL  # 16 segments per tile
NPART = BULK * SEGS_PER_FILL  # 120 partitions used

_PROGRAM_CACHE: dict = {}


def _build_program(n_seg_core: int, count: int) -> bacc.Bacc:
    """Bass program for one core: n_seg_core segments, max over first
    `count` rows of each. Requires SEG_LEN - J < count <= SEG_LEN."""
    assert SEG_LEN - J < count <= SEG_LEN
    rows = n_seg_core * SEG_LEN
    n_tiles = n_seg_core // SEGS_PER_TILE
    n_slot = n_seg_core // 128  # tail chunks of 128 segments
    has_tail = count < SEG_LEN
    f32 = mybir.dt.float32
    bf16 = mybir.dt.bfloat16

    nc = bacc.Bacc("TRN2", target_bir_lowering=False, debug=False)
    x_in = nc.dram_tensor("x", [rows, D], f32, kind="ExternalInput")
    out_t = nc.dram_tensor("out", [n_seg_core, D], f32, kind="ExternalOutput")

    # run index = ((t*FILLS + f)*SEGS_PER_FILL + l)*RUNS + i
    x_bulk = x_in.rearrange(
        "(t f l i j) d -> t l i f j d", f=FILLS, l=SEGS_PER_FILL, i=RUNS, j=J
    )
    # tail view: partition p, chunk a -> rows of segment a*128+p
    x_tail = x_in.rearrange("(a p q) d -> p a q d", p=128, q=SEG_LEN)

    rings = [nc.sync, nc.scalar]

    with tile.TileContext(nc) as tc:
        with (
            tc.tile_pool(name="io", bufs=4) as io_pool,
            tc.tile_pool(name="work", bufs=3) as work_pool,
            tc.tile_pool(name="tailp", bufs=1) as tail_pool,
            tc.tile_pool(name="psum", bufs=4, space="PSUM") as psum_pool,
            tc.tile_pool(name="psum2", bufs=2, space="PSUM") as psum_pool2,
            tc.tile_pool(name="consts", bufs=1) as consts,
        ):
            ident_f = consts.tile([128, 128], f32)
            make_identity(nc, ident_f)
            outbuf = consts.tile([128, n_seg_core], f32)
            if has_tail:
                acc = consts.tile([128, n_seg_core], f32, tag="accbuf")
            else:
                acc = outbuf

            for t in range(n_tiles):
                tl = io_pool.tile([NPART, FILLS, J, D], f32, tag="tl")
                for l in range(SEGS_PER_FILL):
                    rings[(t + l) % 2].dma_start(
                        out=tl[BULK * l : BULK * l + BULK],
                        in_=x_bulk[t, l, :BULK],
                    )

                # fold 32 -> 1 along j: f32 -> bf16, bf16 tree, bf16 -> f32
                w = work_pool.tile([NPART, FILLS, J // 2, D], bf16, tag="w")
                nc.vector.tensor_max(
                    out=w, in0=tl[:, :, : J // 2], in1=tl[:, :, J // 2 :]
                )
                k = J // 2
                while k > 2:
                    k //= 2
                    nc.vector.tensor_max(
                        out=w[:, :, :k], in0=w[:, :, :k], in1=w[:, :, k : 2 * k]
                    )
                wf = work_pool.tile([NPART, FILLS, D], f32, tag="wf")
                nc.vector.tensor_max(out=wf, in0=w[:, :, 0], in1=w[:, :, 1])

                # per fill: transpose [120, 128] -> [128, 120], reduce segs
                for f in range(FILLS):
                    gf = t * SEGS_PER_TILE + f * SEGS_PER_FILL
                    pt = psum_pool.tile([128, SEGS_PER_FILL, BULK], f32, tag="pt")
                    nc.tensor.transpose(
                        pt.rearrange("p a b -> p (a b)"),
                        wf[:, f],
                        ident_f[:NPART, :NPART],
                    )
                    nc.vector.reduce_max(
                        out=acc[:, gf : gf + SEGS_PER_FILL],
                        in_=pt,
                        axis=mybir.AxisListType.X,
                    )

            if has_tail:
                # fold the valid tail rows of every segment, then join
                taibuf = consts.tile([128, n_seg_core], f32)
                for a in range(n_slot):
                    tt = tail_pool.tile([128, J, D], f32, tag="tt")
                    rings[a % 2].dma_start(
                        out=tt, in_=x_tail[:, a, count - J : count]
                    )
                    wt = tail_pool.tile([128, J // 2, D], bf16, tag="wt")
                    nc.vector.tensor_max(
                        out=wt, in0=tt[:, : J // 2], in1=tt[:, J // 2 :]
                    )
                    k = J // 2
                    while k > 2:
                        k //= 2
                        nc.vector.tensor_max(
                            out=wt[:, :k], in0=wt[:, :k], in1=wt[:, k : 2 * k]
                        )
                    wtf = tail_pool.tile([128, D], f32, tag="wtf")
                    nc.vector.tensor_max(out=wtf, in0=wt[:, 0], in1=wt[:, 1])
                    ptt = psum_pool2.tile([128, 128], f32, tag="ptt")
                    nc.tensor.transpose(ptt, wtf, ident_f)
                    nc.scalar.copy(taibuf[:, a * 128 : (a + 1) * 128], ptt)
                nc.vector.tensor_max(out=outbuf, in0=acc, in1=taibuf)

            # outbuf is [128 d, n_seg_core]; transpose back to [seg, d]
            for c in range(n_seg_core // 128):
                pt = psum_pool2.tile([128, 128], f32, tag="ot_ps")
                nc.tensor.transpose(pt, outbuf[:, c * 128 : (c + 1) * 128], ident_f)
                ot = io_pool.tile([128, 128], f32, tag="ot")
                nc.scalar.copy(ot, pt)
                nc.sync.dma_start(out=out_t[c * 128 : (c + 1) * 128, :], in_=ot)
    nc.compile()
    return nc


def _numpy_fallback(x: np.ndarray, sizes: np.ndarray, w: int) -> np.ndarray:
    ends = np.cumsum(sizes)
    starts = ends - sizes
    out = np.full((sizes.shape[0], x.shape[1]), -np.inf, dtype=np.float32)
    for i in range(sizes.shape[0]):
        c = int(sizes[i]) - w + 1
        if c > 0:
            out[i] = x[int(starts[i]) : int(starts[i]) + c].max(axis=0)
    return out


def kernel(x, sizes, window_size) -> np.ndarray:
    x = np.ascontiguousarray(np.asarray(x, dtype=np.float32))
    sizes = np.asarray(sizes)
    w = int(np.asarray(window_size))
    n_seg = sizes.shape[0]
    count = SEG_LEN - w + 1

    uniform = (
        x.ndim == 2
        and x.shape[1] == D
        and bool((sizes == SEG_LEN).all())
        and x.shape[0] == n_seg * SEG_LEN
        and n_seg % (N_CORES * SEGS_PER_TILE) == 0
        and (n_seg // N_CORES) % 128 == 0
        and SEG_LEN - J < count <= SEG_LEN
    )
    if not uniform:
        return _numpy_fallback(x, sizes, w)

    n_seg_core = n_seg // N_CORES
    key = (n_seg_core, count)
    if key not in _PROGRAM_CACHE:
        _PROGRAM_CACHE[key] = _build_program(n_seg_core, count)
    nc = _PROGRAM_CACHE[key]

    shards = np.split(x, N_CORES, axis=0)
    in_maps = [{"x": s} for s in shards]
    res = run_bass_kernel_spmd(nc, in_maps, core_ids=list(range(N_CORES)))
    return np.concatenate([r["out"] for r in res.results], axis=0)


# revision 14
# speedup vs baseline: 2.4078x; 2.4078x over previous
"""Segment-prefix max kernel for Trainium2 (8 NeuronCores, SPMD).

Problem: x [1048576, 128] f32, 2048 uniform segments of 512 rows each;
out[i, :] = max over the first (512 - window_size + 1) rows of segment i.

Strategy (memory-bound, streams ~512 MiB from HBM at the device wall):
  - Shard segments across 8 cores: core c gets rows [c*131072, (c+1)*131072)
    and produces out rows [c*256, (c+1)*256). No cross-core communication.
  - Per core, 16 tiles of 4 MiB (16 segments): partition p holds runs p and
    128+p of the tile's 256 consecutive 32-row runs — one 16 KiB contiguous
    DMA run per partition per fill (vs 2 KiB naive). One big DMA per tile,
    alternating between the sync and scalar HWDGE rings (each ring is a
    serial ~200 GB/s pipe, so transfers must be large and few).
  - Run 15 of each segment contains the invalid window tail (rows >= count)
    and is EXCLUDED from the reduce. The valid tail rows [count-32, count)
    of all segments are loaded after the tile loop (2 x 2 MiB), folded, and
    joined with one final max — keeping the pipeline ramp clean.
  - The 32 -> 1 fold along the free axis runs on DVE as a binary tree; the
    first level reads f32 and writes bf16, middle levels run in bf16 at 2x
    DVE throughput, the last level emits f32 (rel tolerance 2e-2 >> bf16's
    ~4e-3 rounding).
  - Cross-partition max (each segment = 16 consecutive partitions of one
    fill) goes through a PE transpose (identity matmul into PSUM) and one
    DVE reduce_max over each segment's first 15 columns into a
    [128 d, n_seg] column accumulator.
  - Final columns are PE-transposed back to row-major [n_seg, 128] chunks
    and DMA'd out.
"""

import numpy as np

import concourse.bacc as bacc
import concourse.bass as bass
import concourse.tile as tile
from concourse import mybir
from concourse.bass_utils import run_bass_kernel_spmd
from concourse.masks import make_identity

N_CORES = 8
SEG_LEN = 512
D = 128
J = 32  # rows per run (16 KiB contiguous DMA run)
RUNS = SEG_LEN // J  # 16 runs per segment
FILLS = 2  # fills per tile; tile = FILLS * 2 MiB
SEGS_PER_FILL = 128 // RUNS  # 8
SEGS_PER_TILE = FILLS * SEGS_PER_FILL  # 16 segments, 4 MiB tiles

_PROGRAM_CACHE: dict = {}


def _build_program(n_seg_core: int, count: int) -> bacc.Bacc:
    """Bass program for one core: n_seg_core segments, max over first
    `count` rows of each. Requires SEG_LEN - J < count <= SEG_LEN."""
    assert SEG_LEN - J < count <= SEG_LEN
    rows = n_seg_core * SEG_LEN
    n_tiles = n_seg_core // SEGS_PER_TILE
    n_slot = n_seg_core // 128  # tail chunks of 128 segments
    has_tail = count < SEG_LEN
    f32 = mybir.dt.float32
    bf16 = mybir.dt.bfloat16

    nc = bacc.Bacc("TRN2", target_bir_lowering=False, debug=False)
    x_in = nc.dram_tensor("x", [rows, D], f32, kind="ExternalInput")
    out_t = nc.dram_tensor("out", [n_seg_core, D], f32, kind="ExternalOutput")

    # tile t, partition p, fill f -> run 256*t + 128*f + p
    x_tile = x_in.rearrange("(t f p j) d -> t p f j d", f=FILLS, p=128, j=J)
    # tail view: partition p, chunk a -> rows of segment a*128+p
    x_tail = x_in.rearrange("(a p q) d -> p a q d", p=128, q=SEG_LEN)

    rings = [nc.sync, nc.scalar]

    with tile.TileContext(nc) as tc:
        with (
            tc.tile_pool(name="io", bufs=4) as io_pool,
            tc.tile_pool(name="work", bufs=3) as work_pool,
            tc.tile_pool(name="tailp", bufs=1) as tail_pool,
            tc.tile_pool(name="psum", bufs=4, space="PSUM") as psum_pool,
            tc.tile_pool(name="psum2", bufs=2, space="PSUM") as psum_pool2,
            tc.tile_pool(name="consts", bufs=1) as consts,
        ):
            ident_f = consts.tile([128, 128], f32)
            make_identity(nc, ident_f)
            outbuf = consts.tile([128, n_seg_core], f32)
            if has_tail:
                acc = consts.tile([128, n_seg_core], f32, tag="accbuf")
            else:
                acc = outbuf

            for t in range(n_tiles):
                tl = io_pool.tile([128, FILLS, J, D], f32, tag="tl")
                g0 = t * SEGS_PER_TILE
                rings[t % 2].dma_start(out=tl, in_=x_tile[t])

                # fold 32 -> 1 along j: f32 -> bf16, bf16 tree, bf16 -> f32
                w = work_pool.tile([128, FILLS, J // 2, D], bf16, tag="w")
                nc.vector.tensor_max(
                    out=w, in0=tl[:, :, : J // 2], in1=tl[:, :, J // 2 :]
                )
                k = J // 2
                while k > 2:
                    k //= 2
                    nc.vector.tensor_max(
                        out=w[:, :, :k], in0=w[:, :, :k], in1=w[:, :, k : 2 * k]
                    )
                wf = work_pool.tile([128, FILLS, D], f32, tag="wf")
                nc.vector.tensor_max(out=wf, in0=w[:, :, 0], in1=w[:, :, 1])

                # per fill: transpose, reduce each segment's valid columns
                for f in range(FILLS):
                    gf = g0 + f * SEGS_PER_FILL
                    pt = psum_pool.tile([128, SEGS_PER_FILL, RUNS], f32, tag="pt")
                    nc.tensor.transpose(
                        pt.rearrange("p a b -> p (a b)"), wf[:, f], ident_f
                    )
                    if has_tail:
                        nc.vector.reduce_max(
                            out=acc[:, gf : gf + SEGS_PER_FILL],
                            in_=pt[:, :, : RUNS - 1],
                            axis=mybir.AxisListType.X,
                        )
                    else:
                        nc.vector.reduce_max(
                            out=acc[:, gf : gf + SEGS_PER_FILL],
                            in_=pt,
                            axis=mybir.AxisListType.X,
                        )

            if has_tail:
                # fold the valid tail rows of every segment, then join
                taibuf = consts.tile([128, n_seg_core], f32)
                for a in range(n_slot):
                    tt = tail_pool.tile([128, J, D], f32, tag="tt")
                    rings[a % 2].dma_start(
                        out=tt, in_=x_tail[:, a, count - J : count]
                    )
                    wt = tail_pool.tile([128, J // 2, D], bf16, tag="wt")
                    nc.vector.tensor_max(
                        out=wt, in0=tt[:, : J // 2], in1=tt[:, J // 2 :]
                    )
                    k = J // 2
                    while k > 2:
                        k //= 2
                        nc.vector.tensor_max(
                            out=wt[:, :k], in0=wt[:, :k], in1=wt[:, k : 2 * k]
                        )
                    wtf = tail_pool.tile([128, D], f32, tag="wtf")
                    nc.vector.tensor_max(out=wtf, in0=wt[:, 0], in1=wt[:, 1])
                    ptt = psum_pool2.tile([128, 128], f32, tag="ptt")
                    nc.tensor.transpose(ptt, wtf, ident_f)
                    nc.scalar.copy(taibuf[:, a * 128 : (a + 1) * 128], ptt)
                nc.vector.tensor_max(out=outbuf, in0=acc, in1=taibuf)

            # outbuf is [128 d, n_seg_core]; transpose back to [seg, d]
            for c in range(n_seg_core // 128):
                pt = psum_pool2.tile([128, 128], f32, tag="ot_ps")
                nc.tensor.transpose(pt, outbuf[:, c * 128 : (c + 1) * 128], ident_f)
                ot = io_pool.tile([128, 128], f32, tag="ot")
                nc.scalar.copy(ot, pt)
                nc.sync.dma_start(out=out_t[c * 128 : (c + 1) * 128, :], in_=ot)
    nc.compile()
    return nc


def _numpy_fallback(x: np.ndarray, sizes: np.ndarray, w: int) -> np.ndarray:
    ends = np.cumsum(sizes)
    starts = ends - sizes
    out = np.full((sizes.shape[0], x.shape[1]), -np.inf, dtype=np.float32)
    for i in range(sizes.shape[0]):
        c = int(sizes[i]) - w + 1
        if c > 0:
            out[i] = x[int(starts[i]) : int(starts[i]) + c].max(axis=0)
    return out


def kernel(x, sizes, window_size) -> np.ndarray:
    x = np.ascontiguousarray(np.asarray(x, dtype=np.float32))
    sizes = np.asarray(sizes)
    w = int(np.asarray(window_size))
    n_seg = sizes.shape[0]
    count = SEG_LEN - w + 1

    uniform = (
        x.ndim == 2
        and x.shape[1] == D
        and bool((sizes == SEG_LEN).all())
        and x.shape[0] == n_seg * SEG_LEN
        and n_seg % (N_CORES * SEGS_PER_TILE) == 0
        and (n_seg // N_CORES) % 128 == 0
        and SEG_LEN - J < count <= SEG_LEN
    )
    if not uniform:
        return _numpy_fallback(x, sizes, w)

    n_seg_core = n_seg // N_CORES
    key = (n_seg_core, count)
    if key not in _PROGRAM_CACHE:
        _PROGRAM_CACHE[key] = _build_program(n_seg_core, count)
    nc = _PROGRAM_CACHE[key]

    shards = np.split(x, N_CORES, axis=0)
    in_maps = [{"x": s} for s in shards]
    res = run_bass_kernel_spmd(nc, in_maps, core_ids=list(range(N_CORES)))
    return np.concatenate([r["out"] for r in res.results], axis=0)


# revision 15
# speedup vs baseline: 2.5385x; 1.0543x over previous
"""Segment-prefix max kernel for Trainium2 (8 NeuronCores, SPMD).

Problem: x [1048576, 128] f32, 2048 uniform segments of 512 rows each;
out[i, :] = max over the first (512 - window_size + 1) rows of segment i.

Strategy (memory-bound, streams ~512 MiB from HBM at the device wall):
  - Shard segments across 8 cores: core c gets rows [c*131072, (c+1)*131072)
    and produces out rows [c*256, (c+1)*256). No cross-core communication.
  - Per core, 16 tiles of 4 MiB (16 segments): partition p holds runs p and
    128+p of the tile's 256 consecutive 32-row runs — one 16 KiB contiguous
    DMA run per partition per fill (vs 2 KiB naive). One big DMA per tile,
    alternating between the sync and scalar HWDGE rings (each ring is a
    serial ~200 GB/s pipe, so transfers must be large and few).
  - Run 15 of each segment contains the invalid window tail (rows >= count)
    and is EXCLUDED from the reduce. The valid tail rows [count-32, count)
    of all segments are loaded after the tile loop (2 x 2 MiB), folded, and
    joined with one final max — keeping the pipeline ramp clean.
  - The 32 -> 1 fold along the free axis runs on DVE as a binary tree; the
    first level reads f32 and writes bf16, middle levels run in bf16 at 2x
    DVE throughput, the last level emits f32 (rel tolerance 2e-2 >> bf16's
    ~4e-3 rounding).
  - Cross-partition max (each segment = 16 consecutive partitions of one
    fill) goes through a PE transpose (identity matmul into PSUM) and one
    DVE reduce_max over each segment's first 15 columns into a
    [128 d, n_seg] column accumulator.
  - Final columns are PE-transposed back to row-major [n_seg, 128] chunks
    and DMA'd out.
"""

import numpy as np

import concourse.bacc as bacc
import concourse.bass as bass
import concourse.tile as tile
from concourse import mybir
from concourse.bass_utils import run_bass_kernel_spmd
from concourse.masks import make_identity

N_CORES = 8
SEG_LEN = 512
D = 128
J = 32  # rows per run (16 KiB contiguous DMA run)
RUNS = SEG_LEN // J  # 16 runs per segment
FILLS = 2  # fills per tile; tile = FILLS * 2 MiB
SEGS_PER_FILL = 128 // RUNS  # 8
SEGS_PER_TILE = FILLS * SEGS_PER_FILL  # 16 segments, 4 MiB tiles

_PROGRAM_CACHE: dict = {}


def _build_program(n_seg_core: int, count: int) -> bacc.Bacc:
    """Bass program for one core: n_seg_core segments, max over first
    `count` rows of each. Requires SEG_LEN - J < count <= SEG_LEN."""
    assert SEG_LEN - J < count <= SEG_LEN
    rows = n_seg_core * SEG_LEN
    n_tiles = n_seg_core // SEGS_PER_TILE
    n_slot = n_seg_core // 128  # tail chunks of 128 segments
    has_tail = count < SEG_LEN
    f32 = mybir.dt.float32
    bf16 = mybir.dt.bfloat16

    nc = bacc.Bacc("TRN2", target_bir_lowering=False, debug=False)
    x_in = nc.dram_tensor("x", [rows, D], f32, kind="ExternalInput")
    out_t = nc.dram_tensor("out", [n_seg_core, D], f32, kind="ExternalOutput")

    # tile t, partition p, fill f -> run 256*t + 128*f + p
    x_tile = x_in.rearrange("(t f p j) d -> t p f j d", f=FILLS, p=128, j=J)
    # tail view: partition p, chunk a -> rows of segment a*128+p
    x_tail = x_in.rearrange("(a p q) d -> p a q d", p=128, q=SEG_LEN)

    rings = [nc.sync, nc.scalar]

    with tile.TileContext(nc) as tc:
        with (
            tc.tile_pool(name="io", bufs=4) as io_pool,
            tc.tile_pool(name="work", bufs=3) as work_pool,
            tc.tile_pool(name="tailp", bufs=1) as tail_pool,
            tc.tile_pool(name="psum", bufs=4, space="PSUM") as psum_pool,
            tc.tile_pool(name="psum2", bufs=2, space="PSUM") as psum_pool2,
            tc.tile_pool(name="consts", bufs=1) as consts,
        ):
            ident_f = consts.tile([128, 128], f32)
            make_identity(nc, ident_f)
            outbuf = consts.tile([128, n_seg_core], f32)
            if has_tail:
                acc = consts.tile([128, n_seg_core], f32, tag="accbuf")
            else:
                acc = outbuf

            if has_tail:
                # fold the valid tail rows of every segment on the gpsimd
                # (SWDGE) ring so the two HWDGE rings carry only bulk tiles
                taibuf = consts.tile([128, n_seg_core], f32)
                tt = tail_pool.tile([128, n_slot, J, D], f32, tag="tt")
                nc.gpsimd.dma_start(out=tt, in_=x_tail[:, :, count - J : count])
                wt = tail_pool.tile([128, n_slot, J // 2, D], bf16, tag="wt")
                nc.vector.tensor_max(
                    out=wt, in0=tt[:, :, : J // 2], in1=tt[:, :, J // 2 :]
                )
                k = J // 2
                while k > 2:
                    k //= 2
                    nc.vector.tensor_max(
                        out=wt[:, :, :k], in0=wt[:, :, :k], in1=wt[:, :, k : 2 * k]
                    )
                wtf = tail_pool.tile([128, n_slot, D], f32, tag="wtf")
                nc.vector.tensor_max(out=wtf, in0=wt[:, :, 0], in1=wt[:, :, 1])
                for a in range(n_slot):
                    ptt = psum_pool2.tile([128, 128], f32, tag="ptt")
                    nc.tensor.transpose(ptt, wtf[:, a], ident_f)
                    nc.scalar.copy(taibuf[:, a * 128 : (a + 1) * 128], ptt)

            for t in range(n_tiles):
                tl = io_pool.tile([128, FILLS, J, D], f32, tag="tl")
                g0 = t * SEGS_PER_TILE
                rings[t % 2].dma_start(out=tl, in_=x_tile[t])

                # fold 32 -> 1 along j: f32 -> bf16, bf16 tree, bf16 -> f32
                w = work_pool.tile([128, FILLS, J // 2, D], bf16, tag="w")
                nc.vector.tensor_max(
                    out=w, in0=tl[:, :, : J // 2], in1=tl[:, :, J // 2 :]
                )
                k = J // 2
                while k > 2:
                    k //= 2
                    nc.vector.tensor_max(
                        out=w[:, :, :k], in0=w[:, :, :k], in1=w[:, :, k : 2 * k]
                    )
                wf = work_pool.tile([128, FILLS, D], f32, tag="wf")
                nc.vector.tensor_max(out=wf, in0=w[:, :, 0], in1=w[:, :, 1])

                # per fill: transpose, reduce each segment's valid columns
                for f in range(FILLS):
                    gf = g0 + f * SEGS_PER_FILL
                    pt = psum_pool.tile([128, SEGS_PER_FILL, RUNS], f32, tag="pt")
                    nc.tensor.transpose(
                        pt.rearrange("p a b -> p (a b)"), wf[:, f], ident_f
                    )
                    if has_tail:
                        nc.vector.reduce_max(
                            out=acc[:, gf : gf + SEGS_PER_FILL],
                            in_=pt[:, :, : RUNS - 1],
                            axis=mybir.AxisListType.X,
                        )
                    else:
                        nc.vector.reduce_max(
                            out=acc[:, gf : gf + SEGS_PER_FILL],
                            in_=pt,
                            axis=mybir.AxisListType.X,
                        )

            if has_tail:
                nc.vector.tensor_max(out=outbuf, in0=acc, in1=taibuf)

            # outbuf is [128 d, n_seg_core]; transpose back to [seg, d]
            ot = io_pool.tile([128, n_seg_core // 128, 128], f32, tag="ot")
            for c in range(n_seg_core // 128):
                pt = psum_pool2.tile([128, 128], f32, tag="ot_ps")
                nc.tensor.transpose(pt, outbuf[:, c * 128 : (c + 1) * 128], ident_f)
                nc.scalar.copy(ot[:, c], pt)
            nc.sync.dma_start(
                out=out_t.rearrange("(c p) d -> p c d", p=128), in_=ot
            )
    nc.compile()
    return nc


def _numpy_fallback(x: np.ndarray, sizes: np.ndarray, w: int) -> np.ndarray:
    ends = np.cumsum(sizes)
    starts = ends - sizes
    out = np.full((sizes.shape[0], x.shape[1]), -np.inf, dtype=np.float32)
    for i in range(sizes.shape[0]):
        c = int(sizes[i]) - w + 1
        if c > 0:
            out[i] = x[int(starts[i]) : int(starts[i]) + c].max(axis=0)
    return out


def kernel(x, sizes, window_size) -> np.ndarray:
    x = np.ascontiguousarray(np.asarray(x, dtype=np.float32))
    sizes = np.asarray(sizes)
    w = int(np.asarray(window_size))
    n_seg = sizes.shape[0]
    count = SEG_LEN - w + 1

    uniform = (
        x.ndim == 2
        and x.shape[1] == D
        and bool((sizes == SEG_LEN).all())
        and x.shape[0] == n_seg * SEG_LEN
        and n_seg % (N_CORES * SEGS_PER_TILE) == 0
        and (n_seg // N_CORES) % 128 == 0
        and SEG_LEN - J < count <= SEG_LEN
    )
    if not uniform:
        return _numpy_fallback(x, sizes, w)

    n_seg_core = n_seg // N_CORES
    key = (n_seg_core, count)
    if key not in _PROGRAM_CACHE:
        _PROGRAM_CACHE[key] = _build_program(n_seg_core, count)
    nc = _PROGRAM_CACHE[key]

    shards = np.split(x, N_CORES, axis=0)
    in_maps = [{"x": s} for s in shards]
    res = run_bass_kernel_spmd(nc, in_maps, core_ids=list(range(N_CORES)))
    return np.concatenate([r["out"] for r in res.results], axis=0)
